# revision 6
# baseline (speedup 1.0000x reference)
"""Correlation kernel (max_disp=1, 9 offsets) for Trainium2, 8 NeuronCores.

Computation (per batch b):
    out[dx*3+dy, i, j] = mean_c( x1[c,i,j] * pad(x2)[c, i+dy, j+dx] )
with B=8, C=512, H=W=96, pad=1 on each spatial side.

Sharding: data-parallel over batch — core b handles batch b. No collectives.

Per-core strategy:
  - C (512) on SBUF partitions, 4 chunks of 128; spatial (96x96=9216) on free dim.
  - Inputs DMA'd with fp32->bf16 cast (SWDGE). x2 lands in a zero-padded
    [128, 98, 98] tile; a second copy shifted by one column (SBUF->SBUF DMA)
    keeps all 9 shifted views 4B-aligned so DVE tensor_mul runs in 2x mode.
  - VectorE: 9 offsets x 4 chunks bf16 elementwise products [128, 9216].
  - TensorE: partition-dim reduction via matmuls whose stationary operand is a
    sliding one-hot-column-of-ones [128, 36] slice (narrow stationary = cheap
    LDWEIGHTS); offset k, 512-col block bb accumulates into row 18*(k%2)+bb of
    PSUM bank k//2 (5 banks of [36, 512]), accumulating all 4 C-chunks.
  - ScalarE (ActE) scales PSUM by 1/512 into SBUF; HWDGE DMAs reshape to
    [9,96,96]. Border memsets hoisted out of the chunk loop (buffers are
    reused, borders stay zero).
"""

import os
import sys

for _p in ("/opt/trn_rl_repo",):
    if os.path.isdir(_p) and _p not in sys.path:
        sys.path.insert(0, _p)

from contextlib import ExitStack

import numpy as np

import concourse.bass as bass
import concourse.mybir as mybir
import concourse.tile as tile
from concourse import bacc
from concourse.bass_utils import run_bass_kernel_spmd

B, C, H, W = 8, 512, 96, 96
NCORES = 8
PW = W + 2          # padded spatial width
HW = H * W          # 9216 free elems
NCHUNK = C // 128   # 4
NBLK = HW // 512    # 18 512-col blocks
F32 = mybir.dt.float32
BF16 = mybir.dt.bfloat16

# offset order: dx=1 (odd-copy dependent) last so the od DMA can complete
# while dx=0/dx=2 products run
K_ORDER = [0, 1, 2, 6, 7, 8, 3, 4, 5]


def _corr_body(ctx: ExitStack, tc: "tile.TileContext", out_t, x1_t, x2_t, nchunk=NCHUNK):
    nc = tc.nc
    mm_chunks = int(os.environ.get("CORR_MM_CHUNKS", str(nchunk)))

    wpool = ctx.enter_context(tc.tile_pool(name="wm", bufs=1))
    x1pool = ctx.enter_context(tc.tile_pool(name="x1", bufs=2))
    # ev/od are explicitly double-buffered (two named tiles, reused across
    # chunks), so the pools themselves hold a single slot per tile.
    evpool = ctx.enter_context(tc.tile_pool(name="ev", bufs=1))
    odpool = ctx.enter_context(tc.tile_pool(name="od", bufs=1))
    prpool = ctx.enter_context(tc.tile_pool(name="pr", bufs=int(os.environ.get("CORR_PROD_BUFS", "4"))))
    pspool = ctx.enter_context(
        tc.tile_pool(name="ps", bufs=1, space=bass.MemorySpace.PSUM)
    )
    outpool = ctx.enter_context(tc.tile_pool(name="ot", bufs=1))

    # Sliding ones-column masters for the M=36 stationary: slice wm*[:, s:s+36]
    # has its all-ones column at local position m0r when s = ones_col - m0r.
    # Two masters (ones at col 36 and 37) keep s even for either parity of
    # m0r, so every LDWEIGHTS source is 4-byte aligned (bf16).
    M = 36
    wmE = wpool.tile([128, 2 * M + 2], BF16)
    nc.vector.memset(wmE[:, :], 0.0)
    nc.vector.memset(wmE[:, M : M + 1], 1.0)
    wmO = wpool.tile([128, 2 * M + 2], BF16)
    nc.vector.memset(wmO[:, :], 0.0)
    nc.vector.memset(wmO[:, M + 1 : M + 2], 1.0)

    def wslice(m0r: int):
        wm, col = (wmE, M) if m0r % 2 == 0 else (wmO, M + 1)
        s = col - m0r
        return wm[:, s : s + M]

    # 5 persistent PSUM banks of [36, 512]; offset k -> bank k//2,
    # row 18*(k%2)+bb.
    psb = [pspool.tile([M, 512], F32, name=f"psb{i}") for i in range(5)]

    x1f = x1_t.ap()  # [512, 96, 96] f32 DRAM
    x2f = x2_t.ap()

    started = [False] * 5
    # last offset (in K_ORDER) hitting each bank
    last_k_for_bank = {}
    for k in K_ORDER:
        last_k_for_bank[k // 2] = k

    # ev/od buffers are reused across chunks (bufs=2) and the interior DMA
    # never touches the borders, so zero them once up front.
    ev_bufs = [evpool.tile([128, PW, PW], BF16, name=f"ev{i}") for i in range(2)]
    od_bufs = [odpool.tile([128, PW, PW], BF16, name=f"od{i}") for i in range(2)]
    for ev in ev_bufs:
        nc.vector.memset(ev[:, 0, :], 0.0)
        nc.vector.memset(ev[:, PW - 1, :], 0.0)
        nc.vector.memset(ev[:, 1 : PW - 1, 0], 0.0)
        nc.vector.memset(ev[:, 1 : PW - 1, PW - 1], 0.0)

    for ch in range(nchunk):
        p0 = ch * 128
        x1bf = x1pool.tile([128, H, W], BF16)
        nc.gpsimd.dma_start(out=x1bf[:, :, :], in_=x1f[p0 : p0 + 128, :, :])

        ev = ev_bufs[ch % 2]
        nc.gpsimd.dma_start(
            out=ev[:, 1 : PW - 1, 1 : PW - 1], in_=x2f[p0 : p0 + 128, :, :]
        )

        # odd copy: flat shift-by-one so dx=1 views are 4B-aligned.
        od = od_bufs[ch % 2]
        ev_flat = ev[:, :, :].rearrange("p a b -> p (a b)")
        od_flat = od[:, :, :].rearrange("p a b -> p (a b)")
        nc.sync.dma_start(out=od_flat[:, 0 : PW * PW - 1], in_=ev_flat[:, 1 : PW * PW])

        for k in K_ORDER:
            dx, dy = divmod(k, 3)
            if dx == 1:
                src, dxx = od, 0
            else:
                src, dxx = ev, dx
            view = src[:, dy : dy + H, dxx : dxx + W]
            prod = prpool.tile([128, H, W], BF16)
            nc.vector.tensor_mul(prod[:, :, :], x1bf[:, :, :], view)

            if ch >= mm_chunks:
                continue
            prod_flat = prod[:, :, :].rearrange("p a b -> p (a b)")
            bank = k // 2
            ps = psb[bank]
            for bb in range(NBLK):
                m0r = 18 * (k % 2) + bb
                st = not started[bank]
                started[bank] = True
                last = (
                    ch == mm_chunks - 1
                    and bb == NBLK - 1
                    and k == last_k_for_bank[bank]
                )
                nc.tensor.matmul(
                    ps[:, :],
                    wslice(m0r),
                    prod_flat[:, bb * 512 : (bb + 1) * 512],
                    start=st,
                    stop=last,
                )

    outs = [outpool.tile([M, 512], F32, name=f"out{i}") for i in range(5)]
    for i in range(5):
        rows = M if i < 4 else 18
        nc.scalar.activation(
            outs[i][:rows, :],
            psb[i][:rows, :],
            mybir.ActivationFunctionType.Copy,
            scale=1.0 / (128 * nchunk),
        )

    outf = out_t.ap()  # [9, 96, 96] f32 DRAM
    for k in range(9):
        src = outs[k // 2][18 * (k % 2) : 18 * (k % 2 + 1), :]
        nc.sync.dma_start(out=outf[k, :, :], in_=src)


_CACHE = {}


def _build(c=C, debug=False):
    key = ("nc", c)
    if key in _CACHE:
        return _CACHE[key]
    nchunk = c // 128
    nc = bacc.Bacc("TRN2", target_bir_lowering=False, debug=debug)
    x1_t = nc.dram_tensor("x_1", [c, H, W], F32, kind="ExternalInput")
    x2_t = nc.dram_tensor("x_2", [c, H, W], F32, kind="ExternalInput")
    out_t = nc.dram_tensor("out", [9, H, W], F32, kind="ExternalOutput")
    with tile.TileContext(nc) as tc, ExitStack() as ctx:
        _corr_body(ctx, tc, out_t, x1_t, x2_t, nchunk=nchunk)
    nc.compile()
    _CACHE[key] = nc
    return nc


def kernel(x_1: np.ndarray, x_2: np.ndarray) -> np.ndarray:
    x_1 = np.ascontiguousarray(np.asarray(x_1), dtype=np.float32)
    x_2 = np.ascontiguousarray(np.asarray(x_2), dtype=np.float32)
    assert x_1.shape == (B, C, H, W) and x_2.shape == (B, C, H, W)
    nc = _build()
    in_maps = [
        {"x_1": x_1[i].copy(), "x_2": x_2[i].copy()} for i in range(NCORES)
    ]
    last_err = None
    for attempt in range(3):
        try:
            res = run_bass_kernel_spmd(nc, in_maps, list(range(NCORES)))
            out = np.stack([res.results[i]["out"] for i in range(NCORES)], axis=0)
            return out.astype(np.float32)
        except Exception as e:  # rare transient device faults — retry
            last_err = e
            import time as _time

            _time.sleep(5.0 * (attempt + 1))
    raise last_err


if __name__ == "__main__":
    rng = np.random.default_rng(0)
    a = rng.standard_normal((B, C, H, W), dtype=np.float32)
    b = rng.standard_normal((B, C, H, W), dtype=np.float32)
    o = kernel(a, b)
    print("out", o.shape, o.dtype, float(np.abs(o).max()))
